# revision 5
# baseline (speedup 1.0000x reference)
"""MoE-routed DeepQNetwork kernel for 8x Trainium2 NeuronCores.

Problem: B=65536 rows, each routed to one of E=8 expert MLPs
(256 -> 64 -> 64 -> 64 -> 64 -> 64 -> 18, ReLU between layers).

Strategy v3 (expert-per-core sharding):
  E == NCORES and the routing is near-uniform (~8192 rows/expert), so core k
  owns ALL rows of expert k, padded to a uniform C = nb*512 columns (nb may
  be odd: pairs of 512-row blocks plus an optional lone block). Every core
  runs the same static program with a SINGLE expert's weights (~180 KB).

  Device (per core, SPMD):
  - PE warm-up: a burst of tiny matmuls right after the preamble releases
    the HAM clock gate (PE idles at 1.2 GHz until ~3.4us of activity) so the
    real matmuls run at 2.4 GHz.
  - x^T arrives as [256, C] fp16; pair p gets a [128, 2048] chunk (two
    blocks x two 128-row input halves). Even pairs + lone block stream on
    the sync HW-DGE queue, odd pairs + weights + bias on the gpsimd SW-DGE
    queue, so two descriptor rings fill SBUF concurrently; pair 0 is split
    into four [128, 512] chunks so the first matmul starts as early as
    possible.
  - L1 per pair: 4 matmuls on PE column groups (block even -> PSUM rows
    0:64, odd -> 64:128), contraction 256 over two accumulating chunks.
    L2-5: one [128,128] block-diag matmul per pair (the same 64x64 weight
    on both diagonals). PSUM/activation tiles cover TWO pairs ([128,1024],
    2 PSUM banks) so each ReLU+bias instruction drains four blocks; the L2
    sweep is interleaved into the x-DMA-paced L1 sweep to keep the PE fed.
    L6 stacks two pairs into one [128,512] PSUM bank (rows 0:18/32:50/
    64:82/96:114). The lone block runs first in every sweep on half-width
    tiles so it never sits in the serial tail.
  - fp32 accumulation in PSUM; ReLU+bias alternates VectorE/ScalarE;
    outputs leave as fp16 via gpsimd-issued DMAs per 4-block group.

  Host: unsort the fp16 outputs back to row order, cast to fp32.
"""

import math
import os

import numpy as np

E = 8
D = 256
H = 64
A = 18
NCORES = 8
BLK = 512  # rows per block (matmul moving-operand free dim / PSUM bank cols)
NWARM = 7  # PE warm-up matmuls (~3us of N=512 streams at the cold clock)

# per-core weight tile [128, WCOLS] fp16 column layout:
#   [0:64)    W1 chunk0 (input dims 0:128)
#   [64:128)  W1 chunk1 (input dims 128:256)
#   [128+128*li : 256+128*li) for li in 0..3: layer 2+li block-diag [128,128]
#             ([0:64,0:64] = W, [64:128,64:128] = W)
#   [640:704) W6 block-diag: [0:64, 0:18] = W6, [64:128, 32:50] = W6
WCOLS = 704

_PROGRAM_CACHE: dict = {}
LAST_RESULTS = None  # test harness can read timing/profile info from here


def _build_program(nb: int):
    """Build the SPMD bass program for nb 512-row blocks per core."""
    import concourse.mybir as mybir
    import concourse.tile as tile
    from concourse import bacc

    f32 = mybir.dt.float32
    f16 = mybir.dt.float16
    Relu = mybir.ActivationFunctionType.Relu
    add = mybir.AluOpType.add
    amax = mybir.AluOpType.max

    npair = nb // 2
    lone = nb % 2  # trailing unpaired block
    ndbl = (npair + 1) // 2  # two-pair tile groups (last may hold one pair)
    ngrp = ndbl + lone  # output column groups in yt

    nc = bacc.Bacc("TRN2")
    xall = nc.declare_dram_parameter(
        "xall", [128, npair * 2048 + lone * 1024], f16, isOutput=False
    )
    wt = nc.declare_dram_parameter("wt", [128, WCOLS], f16, isOutput=False)
    # bias cols 0:5 = b1..b5 (rows 0:64 == rows 64:128); col 5 = b6 at rows
    # 0:18 / 32:50 / 64:82 / 96:114
    bias = nc.declare_dram_parameter("bias", [128, 6], f32, isOutput=False)
    yt = nc.declare_dram_parameter("yt", [128, ngrp * BLK], f16, isOutput=True)

    act_flip = 0

    with tile.TileContext(nc) as tc:
        with (
            tc.tile_pool(name="wpool", bufs=1) as wpool,
            tc.tile_pool(name="xpool", bufs=npair + 4) as xpool,
            tc.tile_pool(name="hpool", bufs=ndbl) as hpool,
            tc.tile_pool(name="opool", bufs=3) as opool,
            tc.tile_pool(name="ppool", bufs=3, space="PSUM") as ppool,
            tc.tile_pool(name="popool", bufs=2, space="PSUM") as popool,
        ):
            # ---- PE warm-up source (memset, no DMA dependence)
            warm_src = wpool.tile([1, BLK], f16, name="warm_src", tag="ws", bufs=1)
            nc.vector.memset(warm_src[:, :], 0.0)

            # ---- DMA issue. gpsimd ring: weights, bias, lone block, odd
            # pairs; sync HW ring: pair0 as four [128,512] chunks, then the
            # remaining even pairs. Order matches compute consumption.
            w_sb = wpool.tile([128, WCOLS], f16, name="w_sb", tag="w", bufs=1)
            nc.gpsimd.dma_start(out=w_sb[:, :], in_=wt[:, :])
            bias_sb = wpool.tile([128, 6], f32, name="bias_sb", tag="bias", bufs=1)
            nc.gpsimd.dma_start(out=bias_sb[:, :], in_=bias[:, :])

            xl = None
            if lone:
                xl = xpool.tile([128, 1024], f16, tag="xl", name="xlone", bufs=1)
                nc.gpsimd.dma_start(
                    out=xl[:, :], in_=xall[:, npair * 2048 : npair * 2048 + 1024]
                )

            # pair 0: four [128,512] chunks in first-use order
            # (b0,c0) (b0,c1) (b1,c0) (b1,c1)
            p0 = []
            for i, off in enumerate((0, 1024, 512, 1536)):
                t = xpool.tile([128, BLK], f16, tag="x0", name=f"x0_{i}", bufs=4)
                nc.sync.dma_start(out=t[:, :], in_=xall[:, off : off + BLK])
                p0.append(t)

            xcs: list = [None] * npair
            for p in range(2, npair, 2):
                xc = xpool.tile([128, 2048], f16, tag="xc", name=f"xc_{p}")
                nc.sync.dma_start(out=xc[:, :], in_=xall[:, p * 2048 : (p + 1) * 2048])
                xcs[p] = xc
            for p in range(1, npair, 2):
                xc = xpool.tile([128, 2048], f16, tag="xc", name=f"xc_{p}")
                nc.gpsimd.dma_start(
                    out=xc[:, :], in_=xall[:, p * 2048 : (p + 1) * 2048]
                )
                xcs[p] = xc

            def x_rhs(p, blk, c):
                if p == 0:
                    return p0[2 * blk + c][:, :]
                return xcs[p][:, c * 1024 + blk * BLK : c * 1024 + (blk + 1) * BLK]

            # ---- PE warm-up burst (writes cycle the ph ring, never read)
            for i in range(NWARM):
                pw = ppool.tile([128, 1024], f32, tag="ph", name=f"warm_{i}")
                nc.tensor.matmul(
                    out=pw[0:64, 0:BLK],
                    lhsT=warm_src[0:1, 0:64],
                    rhs=warm_src[0:1, :],
                    start=True,
                    stop=True,
                )

            def act(out_ap, in_ap, bias_ap, relu):
                nonlocal act_flip
                act_flip ^= 1
                if act_flip:
                    if relu:
                        nc.vector.tensor_scalar(
                            out_ap, in_ap, bias_ap, 0.0, op0=add, op1=amax
                        )
                    else:
                        nc.vector.tensor_scalar(
                            out_ap, in_ap, bias_ap, None, op0=add
                        )
                else:
                    if relu:
                        nc.scalar.activation(out_ap, in_ap, Relu, bias=bias_ap)
                    else:
                        nc.scalar.add(out_ap, in_ap, bias_ap)

            def dbl_pairs(d):
                return [q for q in (2 * d, 2 * d + 1) if q < npair]

            # ---- Layer 1 (lone block first, then two-pair groups) with the
            # L2 sweep interleaved two groups behind to fill x-DMA wait gaps.
            h1l = None
            hl_cur = None
            if lone:
                phl = ppool.tile([128, 1024], f32, tag="ph", name="ph1_l")
                for c in (0, 1):
                    nc.tensor.matmul(
                        out=phl[0:64, 0:BLK],
                        lhsT=w_sb[:, c * H : (c + 1) * H],
                        rhs=xl[:, c * BLK : (c + 1) * BLK],
                        start=(c == 0),
                        stop=(c == 1),
                    )
                h1l = hpool.tile([64, BLK], f16, tag="hl1", name="h1_l", bufs=1)
                act(h1l[:, :], phl[0:64, 0:BLK], bias_sb[0:64, 0:1], True)
                hl_cur = h1l

            h1s = []

            def emit_l1(d):
                ph1 = ppool.tile([128, 1024], f32, tag="ph", name=f"ph1_{d}")
                for k, p in enumerate(dbl_pairs(d)):
                    co = k * BLK
                    for blk, colr in ((0, slice(0, 64)), (1, slice(64, 128))):
                        for c in (0, 1):
                            nc.tensor.matmul(
                                out=ph1[colr, co : co + BLK],
                                lhsT=w_sb[:, c * H : (c + 1) * H],
                                rhs=x_rhs(p, blk, c),
                                start=(c == 0),
                                stop=(c == 1),
                            )
                w = len(dbl_pairs(d)) * BLK
                h1 = hpool.tile([128, w], f16, tag="h1", name=f"h1_{d}")
                act(h1[:, :], ph1[:, 0:w], bias_sb[:, 0:1], True)
                h1s.append(h1)

            h2s = []
            h2l = None

            def emit_l2(d):
                nonlocal h2l
                if d == -1:  # lone block
                    ph = ppool.tile([128, 1024], f32, tag="ph", name="ph2_l")
                    nc.tensor.matmul(
                        out=ph[0:64, 0:BLK],
                        lhsT=w_sb[0:64, 128:192],
                        rhs=hl_cur[:, :],
                        start=True,
                        stop=True,
                    )
                    h2l = hpool.tile([64, BLK], f16, tag="hl2", name="h2_l", bufs=1)
                    act(h2l[:, :], ph[0:64, 0:BLK], bias_sb[0:64, 1:2], True)
                    return
                ph = ppool.tile([128, 1024], f32, tag="ph", name=f"ph2_{d}")
                w = len(dbl_pairs(d)) * BLK
                for k, p in enumerate(dbl_pairs(d)):
                    co = k * BLK
                    nc.tensor.matmul(
                        out=ph[:, co : co + BLK],
                        lhsT=w_sb[:, 128:256],
                        rhs=h1s[d][:, co : co + BLK],
                        start=True,
                        stop=True,
                    )
                h2 = hpool.tile([128, w], f16, tag="h2", name=f"h2_{d}")
                act(h2[:, :], ph[:, 0:w], bias_sb[:, 1:2], True)
                h2s.append(h2)

            # interleave: L1 groups with L2 lagging two groups behind
            l2q = ([-1] if lone else []) + list(range(ndbl))
            l2i = 0
            for d in range(ndbl):
                emit_l1(d)
                if d >= 2:
                    emit_l2(l2q[l2i])
                    l2i += 1
            while l2i < len(l2q):
                emit_l2(l2q[l2i])
                l2i += 1

            def emit_l6(g, h5s, h5l):
                # group g = pairs 2g (PSUM rows 0:64) and 2g+1 (64:128);
                # g == -1 is the lone block (rows 0:32, output col ndbl)
                if g == -1:
                    po = popool.tile([32, BLK], f32, tag="po", name="po_l")
                    nc.tensor.matmul(
                        out=po[:, :],
                        lhsT=w_sb[0:64, 640:672],
                        rhs=h5l[:, :],
                        start=True,
                        stop=True,
                    )
                    o = opool.tile([32, BLK], f16, tag="og", name="o_l")
                    act(o[:, :], po[:, :], bias_sb[0:32, 5:6], False)
                    nc.gpsimd.dma_start(
                        out=yt[0:32, ndbl * BLK : (ndbl + 1) * BLK], in_=o[:, :]
                    )
                    return
                pairs = dbl_pairs(g)
                rows = 64 * len(pairs)
                po = popool.tile([rows, BLK], f32, tag="po", name=f"po_{g}")
                for k, q in enumerate(pairs):
                    nc.tensor.matmul(
                        out=po[64 * k : 64 * (k + 1), :],
                        lhsT=w_sb[:, 640:704],
                        rhs=h5s[g][:, k * BLK : (k + 1) * BLK],
                        start=True,
                        stop=True,
                    )
                o = opool.tile([rows, BLK], f16, tag="og", name=f"o_{g}")
                act(o[:, :], po[:, :], bias_sb[0:rows, 5:6], False)
                nc.gpsimd.dma_start(
                    out=yt[0:rows, g * BLK : (g + 1) * BLK], in_=o[:, :]
                )

            # ---- Layers 3-5 sweeps (lone first), L6 trailing inside L5
            hcur = h2s
            hlp = h2l
            for li in (1, 2, 3):
                wc = 128 + li * 128
                hnext = []
                hln = None
                if lone:
                    ph = ppool.tile([128, 1024], f32, tag="ph", name=f"ph{li+2}_l")
                    nc.tensor.matmul(
                        out=ph[0:64, 0:BLK],
                        lhsT=w_sb[0:64, wc : wc + 64],
                        rhs=hlp[:, :],
                        start=True,
                        stop=True,
                    )
                    hln = hpool.tile(
                        [64, BLK], f16, tag=f"hl{li+2}", name=f"h{li+2}_l", bufs=1
                    )
                    act(hln[:, :], ph[0:64, 0:BLK], bias_sb[0:64, li + 1 : li + 2], True)
                pend6 = []
                if li == 3 and lone:
                    pend6.append(-1)  # lone L6 after first group's L5
                for d in range(ndbl):
                    ph = ppool.tile([128, 1024], f32, tag="ph", name=f"ph{li+2}_{d}")
                    w = len(dbl_pairs(d)) * BLK
                    for k, p in enumerate(dbl_pairs(d)):
                        co = k * BLK
                        nc.tensor.matmul(
                            out=ph[:, co : co + BLK],
                            lhsT=w_sb[:, wc : wc + 128],
                            rhs=hcur[d][:, co : co + BLK],
                            start=True,
                            stop=True,
                        )
                    h = hpool.tile([128, w], f16, tag=f"h{li+2}", name=f"h{li+2}_{d}")
                    act(h[:, :], ph[:, 0:w], bias_sb[:, li + 1 : li + 2], True)
                    hnext.append(h)
                    if li == 3:
                        # emit pending L6 groups (lagging one L5 group)
                        for g in pend6:
                            emit_l6(g, hnext, hln)
                        pend6 = [d]
                if li == 3:
                    for g in pend6:
                        emit_l6(g, hnext, hln)
                hcur = hnext
                hlp = hln

    nc.compile()
    return nc


def _get_program(nb: int):
    if nb not in _PROGRAM_CACHE:
        _PROGRAM_CACHE[nb] = _build_program(nb)
    return _PROGRAM_CACHE[nb]


def _prepare(state, rm_state, W1, b1, W2, b2, W3, b3, W4, b4, W5, b5, W6, b6):
    state = np.ascontiguousarray(np.asarray(state, dtype=np.float32))
    rm = np.asarray(rm_state).reshape(-1).astype(np.int64)
    Ws = [np.asarray(w, dtype=np.float32) for w in (W1, W2, W3, W4, W5, W6)]
    bs = [np.asarray(b, dtype=np.float32) for b in (b1, b2, b3, b4, b5, b6)]
    B = state.shape[0]
    X = state.reshape(B, D)

    # ---- host-side routing: all rows of expert k go to core k
    order = np.argsort(rm, kind="stable")
    counts = np.bincount(rm, minlength=E)
    nb = max(2, math.ceil(counts.max() / BLK))
    C = nb * BLK
    npair = nb // 2
    lone = nb % 2
    ndbl = (npair + 1) // 2
    ngrp = ndbl + lone
    csum = np.zeros(E, dtype=np.int64)
    csum[1:] = np.cumsum(counts)[:-1]
    sorted_expert = rm[order]
    pos_sorted = sorted_expert * C + (np.arange(B) - csum[sorted_expert])

    Xp = np.zeros((E * C, D), np.float16)
    Xp[pos_sorted] = X[order].astype(np.float16)

    W16 = [w.astype(np.float16) for w in Ws]

    in_maps = []
    for core in range(E):
        xt = Xp[core * C : (core + 1) * C].T  # [D, C] fp16 view
        # pairs: interleave the two 128-row halves per pair -> [128, 2048]
        parts = [
            xt[:, : npair * 1024]
            .reshape(2, 128, npair, 2 * BLK)
            .transpose(1, 2, 0, 3)
            .reshape(128, npair * 4 * BLK)
        ]
        if lone:
            xl = xt[:, npair * 1024 :].reshape(2, 128, BLK)
            parts.append(xl[0])
            parts.append(xl[1])
        xint = np.ascontiguousarray(np.concatenate(parts, axis=1))

        wh = np.zeros((128, WCOLS), np.float16)
        wh[:, 0:H] = W16[0][core, 0:128, :]
        wh[:, H : 2 * H] = W16[0][core, 128:256, :]
        for li in range(4):
            wc = 128 + li * 128
            wh[0:64, wc : wc + H] = W16[li + 1][core]
            wh[64:128, wc + H : wc + 128] = W16[li + 1][core]
        wh[0:64, 640 : 640 + A] = W16[5][core]
        wh[64:128, 672 : 672 + A] = W16[5][core]

        bh = np.zeros((128, 6), np.float32)
        for li in range(5):
            bh[0:64, li] = bs[li][core]
            bh[64:128, li] = bs[li][core]
        for r0 in (0, 32, 64, 96):
            bh[r0 : r0 + A, 5] = bs[5][core]

        in_maps.append({"xall": xint, "wt": wh, "bias": bh})

    meta = dict(
        B=B,
        C=C,
        nb=nb,
        npair=npair,
        lone=lone,
        ndbl=ndbl,
        ngrp=ngrp,
        order=order,
        pos_sorted=pos_sorted,
    )
    return in_maps, meta


def _finalize(results, meta):
    """results: list (per core) of dicts with 'yt' [128, ngrp*BLK] fp16."""
    B, C, nb, npair, lone, ndbl = (
        meta[k] for k in ("B", "C", "nb", "npair", "lone", "ndbl")
    )
    Yp = np.zeros((E * C, A), np.float32)
    for core in range(E):
        ytc = results[core]["yt"].astype(np.float32)
        for g in range(ndbl):
            cols = slice(g * BLK, (g + 1) * BLK)
            for k, q in enumerate((2 * g, 2 * g + 1)):
                if q >= npair:
                    continue
                dst = core * C + 2 * q * BLK
                r0 = 64 * k
                Yp[dst : dst + BLK] = ytc[r0 : r0 + A, cols].T
                Yp[dst + BLK : dst + 2 * BLK] = ytc[r0 + 32 : r0 + 32 + A, cols].T
        if lone:
            cols = slice(ndbl * BLK, (ndbl + 1) * BLK)
            dst = core * C + (nb - 1) * BLK
            Yp[dst : dst + BLK] = ytc[0:A, cols].T

    y = np.zeros((B, A), np.float32)
    y[meta["order"]] = Yp[meta["pos_sorted"]]
    return y


def kernel(state, rm_state, W1, b1, W2, b2, W3, b3, W4, b4, W5, b5, W6, b6):
    global LAST_RESULTS
    from concourse.bass_utils import run_bass_kernel_spmd

    in_maps, meta = _prepare(
        state, rm_state, W1, b1, W2, b2, W3, b3, W4, b4, W5, b5, W6, b6
    )
    nc = _get_program(meta["nb"])
    trace = bool(os.environ.get("KERNEL_TRACE"))
    res = run_bass_kernel_spmd(nc, in_maps, core_ids=list(range(NCORES)), trace=trace)
    LAST_RESULTS = res
    return _finalize(res.results, meta)


# revision 8
# speedup vs baseline: 1.1415x; 1.1415x over previous
"""MoE-routed DeepQNetwork kernel for 8x Trainium2 NeuronCores.

Problem: B=65536 rows, each routed to one of E=8 expert MLPs
(256 -> 64 -> 64 -> 64 -> 64 -> 64 -> 18, ReLU between layers).

Strategy v3 (expert-per-core sharding):
  E == NCORES and the routing is near-uniform (~8192 rows/expert), so core k
  owns ALL rows of expert k, padded to a uniform C = nb*512 columns (nb may
  be odd: pairs of 512-row blocks plus an optional lone block). Every core
  runs the same static program with a SINGLE expert's weights (~180 KB).

  Device (per core, SPMD):
  - PE warm-up: a burst of tiny matmuls right after the preamble releases
    the HAM clock gate (PE idles at 1.2 GHz until ~3.4us of activity) so the
    real matmuls run at 2.4 GHz.
  - x^T arrives as [256, C] fp16; pair p gets a [128, 2048] chunk (two
    blocks x two 128-row input halves). Even pairs + lone block stream on
    the sync HW-DGE queue, odd pairs + weights + bias on the gpsimd SW-DGE
    queue, so two descriptor rings fill SBUF concurrently; pair 0 is split
    into four [128, 512] chunks so the first matmul starts as early as
    possible.
  - L1 per pair: 4 matmuls on PE column groups (block even -> PSUM rows
    0:64, odd -> 64:128), contraction 256 over two accumulating chunks.
    L2-5: one [128,128] block-diag matmul per pair (the same 64x64 weight
    on both diagonals). PSUM/activation tiles cover TWO pairs ([128,1024],
    2 PSUM banks) so each ReLU+bias instruction drains four blocks; the L2
    sweep is interleaved into the x-DMA-paced L1 sweep to keep the PE fed.
    L6 stacks two pairs into one [128,512] PSUM bank (rows 0:18/32:50/
    64:82/96:114). The lone block runs first in every sweep on half-width
    tiles so it never sits in the serial tail.
  - fp32 accumulation in PSUM; ReLU+bias alternates VectorE/ScalarE;
    outputs leave as fp16 via gpsimd-issued DMAs per 4-block group.

  Host: unsort the fp16 outputs back to row order, cast to fp32.
"""

import math
import os

import numpy as np

E = 8
D = 256
H = 64
A = 18
NCORES = 8
BLK = 512  # rows per block (matmul moving-operand free dim / PSUM bank cols)
NWARM = 9  # PE warm-up matmuls (~4.5us of N=512 streams at the cold clock)

# per-core weight tile [128, WCOLS] fp16 column layout:
#   [0:64)    W1 chunk0 (input dims 0:128)
#   [64:128)  W1 chunk1 (input dims 128:256)
#   [128+128*li : 256+128*li) for li in 0..3: layer 2+li block-diag [128,128]
#             ([0:64,0:64] = W, [64:128,64:128] = W)
#   [640:704) W6 block-diag: [0:64, 0:18] = W6, [64:128, 32:50] = W6
WCOLS = 704

_PROGRAM_CACHE: dict = {}
LAST_RESULTS = None  # test harness can read timing/profile info from here


def _build_program(nb: int):
    """Build the SPMD bass program for nb 512-row blocks per core."""
    import concourse.mybir as mybir
    import concourse.tile as tile
    from concourse import bacc

    f32 = mybir.dt.float32
    f16 = mybir.dt.float16
    Relu = mybir.ActivationFunctionType.Relu
    add = mybir.AluOpType.add
    amax = mybir.AluOpType.max

    npair = nb // 2
    lone = nb % 2  # trailing unpaired block
    ndbl = (npair + 1) // 2  # two-pair tile groups (last may hold one pair)
    ngrp = ndbl + lone  # output column groups in yt

    nc = bacc.Bacc("TRN2")
    xall = nc.declare_dram_parameter(
        "xall", [128, npair * 2048 + lone * 1024], f16, isOutput=False
    )
    wt = nc.declare_dram_parameter("wt", [128, WCOLS], f16, isOutput=False)
    # bias cols 0:5 = b1..b5 (rows 0:64 == rows 64:128); col 5 = b6 at rows
    # 0:18 / 32:50 / 64:82 / 96:114
    bias = nc.declare_dram_parameter("bias", [128, 6], f32, isOutput=False)
    yt = nc.declare_dram_parameter("yt", [128, ngrp * BLK], f16, isOutput=True)

    act_flip = 0

    with tile.TileContext(nc) as tc:
        with (
            tc.tile_pool(name="wpool", bufs=1) as wpool,
            tc.tile_pool(name="xpool", bufs=npair + 4) as xpool,
            tc.tile_pool(name="hpool", bufs=ndbl) as hpool,
            tc.tile_pool(name="opool", bufs=3) as opool,
            tc.tile_pool(name="ppool", bufs=3, space="PSUM") as ppool,
            tc.tile_pool(name="popool", bufs=2, space="PSUM") as popool,
        ):
            # ---- PE warm-up source (memset, no DMA dependence)
            warm_src = wpool.tile([1, BLK], f16, name="warm_src", tag="ws", bufs=1)
            nc.vector.memset(warm_src[:, :], 0.0)

            # ---- DMA issue. Splitting x across two descriptor rings measured
            # SLOWER (early aggregate rate collapsed), so ALL x rides the sync
            # HW-DGE ring in exact consumption order; the tiny weight/bias
            # tensors ride the scalar HW-DGE ring in parallel, and gpsimd
            # keeps only the output stores.
            w_sb = wpool.tile([128, WCOLS], f16, name="w_sb", tag="w", bufs=1)
            nc.scalar.dma_start(out=w_sb[:, :], in_=wt[:, :])
            bias_sb = wpool.tile([128, 6], f32, name="bias_sb", tag="bias", bufs=1)
            nc.scalar.dma_start(out=bias_sb[:, :], in_=bias[:, :])

            # pair 0: four [128,512] chunks in first-use order
            # (b0,c0) (b0,c1) (b1,c0) (b1,c1)
            p0 = []
            for i, off in enumerate((0, 1024, 512, 1536)):
                t = xpool.tile([128, BLK], f16, tag="x0", name=f"x0_{i}", bufs=4)
                nc.sync.dma_start(out=t[:, :], in_=xall[:, off : off + BLK])
                p0.append(t)

            xl = None
            if lone:
                xl = xpool.tile([128, 1024], f16, tag="xl", name="xlone", bufs=1)
                nc.sync.dma_start(
                    out=xl[:, :], in_=xall[:, npair * 2048 : npair * 2048 + 1024]
                )

            xcs: list = [None] * npair
            for p in range(1, npair):
                xc = xpool.tile([128, 2048], f16, tag="xc", name=f"xc_{p}")
                nc.sync.dma_start(out=xc[:, :], in_=xall[:, p * 2048 : (p + 1) * 2048])
                xcs[p] = xc

            def x_rhs(p, blk, c):
                if p == 0:
                    return p0[2 * blk + c][:, :]
                return xcs[p][:, c * 1024 + blk * BLK : c * 1024 + (blk + 1) * BLK]

            # ---- PE warm-up burst (writes cycle the ph ring, never read)
            for i in range(NWARM):
                pw = ppool.tile([128, 1024], f32, tag="ph", name=f"warm_{i}")
                nc.tensor.matmul(
                    out=pw[0:64, 0:BLK],
                    lhsT=warm_src[0:1, 0:64],
                    rhs=warm_src[0:1, :],
                    start=True,
                    stop=True,
                )

            def act(out_ap, in_ap, bias_ap, relu):
                nonlocal act_flip
                act_flip ^= 1
                if act_flip:
                    if relu:
                        nc.vector.tensor_scalar(
                            out_ap, in_ap, bias_ap, 0.0, op0=add, op1=amax
                        )
                    else:
                        nc.vector.tensor_scalar(
                            out_ap, in_ap, bias_ap, None, op0=add
                        )
                else:
                    if relu:
                        nc.scalar.activation(out_ap, in_ap, Relu, bias=bias_ap)
                    else:
                        nc.scalar.add(out_ap, in_ap, bias_ap)

            def dbl_pairs(d):
                return [q for q in (2 * d, 2 * d + 1) if q < npair]

            def emit_fill(i):
                # keep-warm filler: the HAM clock gate drops the PE back to
                # 1.2 GHz after ~3.4us of idle, so bridge early x-DMA gaps
                pw = ppool.tile([128, 1024], f32, tag="ph", name=f"fill_{i}")
                nc.tensor.matmul(
                    out=pw[0:64, 0:BLK],
                    lhsT=warm_src[0:1, 0:64],
                    rhs=warm_src[0:1, :],
                    start=True,
                    stop=True,
                )

            h1s = [None] * ndbl
            h1l = None

            def emit_l1(d):
                nonlocal h1l
                if d == -1:  # lone block
                    phl = ppool.tile([128, 1024], f32, tag="ph", name="ph1_l")
                    for c in (0, 1):
                        nc.tensor.matmul(
                            out=phl[0:64, 0:BLK],
                            lhsT=w_sb[:, c * H : (c + 1) * H],
                            rhs=xl[:, c * BLK : (c + 1) * BLK],
                            start=(c == 0),
                            stop=(c == 1),
                        )
                    h1l = hpool.tile([64, BLK], f16, tag="hl1", name="h1_l", bufs=1)
                    act(h1l[:, :], phl[0:64, 0:BLK], bias_sb[0:64, 0:1], True)
                    return
                ph1 = ppool.tile([128, 1024], f32, tag="ph", name=f"ph1_{d}")
                for k, p in enumerate(dbl_pairs(d)):
                    co = k * BLK
                    for blk, colr in ((0, slice(0, 64)), (1, slice(64, 128))):
                        for c in (0, 1):
                            nc.tensor.matmul(
                                out=ph1[colr, co : co + BLK],
                                lhsT=w_sb[:, c * H : (c + 1) * H],
                                rhs=x_rhs(p, blk, c),
                                start=(c == 0),
                                stop=(c == 1),
                            )
                w = len(dbl_pairs(d)) * BLK
                h1 = hpool.tile([128, w], f16, tag="h1", name=f"h1_{d}")
                act(h1[:, :], ph1[:, 0:w], bias_sb[:, 0:1], True)
                h1s[d] = h1

            h2s = [None] * ndbl
            h2l = None

            def emit_l2(d):
                nonlocal h2l
                if d == -1:  # lone block
                    ph = ppool.tile([128, 1024], f32, tag="ph", name="ph2_l")
                    nc.tensor.matmul(
                        out=ph[0:64, 0:BLK],
                        lhsT=w_sb[0:64, 128:192],
                        rhs=h1l[:, :],
                        start=True,
                        stop=True,
                    )
                    h2l = hpool.tile([64, BLK], f16, tag="hl2", name="h2_l", bufs=1)
                    act(h2l[:, :], ph[0:64, 0:BLK], bias_sb[0:64, 1:2], True)
                    return
                ph = ppool.tile([128, 1024], f32, tag="ph", name=f"ph2_{d}")
                w = len(dbl_pairs(d)) * BLK
                for k, p in enumerate(dbl_pairs(d)):
                    co = k * BLK
                    nc.tensor.matmul(
                        out=ph[:, co : co + BLK],
                        lhsT=w_sb[:, 128:256],
                        rhs=h1s[d][:, co : co + BLK],
                        start=True,
                        stop=True,
                    )
                h2 = hpool.tile([128, w], f16, tag="h2", name=f"h2_{d}")
                act(h2[:, :], ph[:, 0:w], bias_sb[:, 1:2], True)
                h2s[d] = h2

            # L1/L2 processing order = DMA arrival order: dbl0 (pair0 split +
            # pair1), lone block, then remaining dbls; L2 lags two units, and
            # two keep-warm fillers bridge the earliest (DMA-paced) gaps.
            units = [0] + ([-1] if lone else []) + list(range(1, ndbl))
            l2i = 0
            for i, u in enumerate(units):
                emit_l1(u)
                if i < 2:
                    emit_fill(i)
                if i >= 2:
                    emit_l2(units[l2i])
                    l2i += 1
            while l2i < len(units):
                emit_l2(units[l2i])
                l2i += 1

            def emit_l6(g, h5s, h5l):
                # group g = pairs 2g (PSUM rows 0:64) and 2g+1 (64:128);
                # g == -1 is the lone block (rows 0:32, output col ndbl)
                if g == -1:
                    po = popool.tile([32, BLK], f32, tag="po", name="po_l")
                    nc.tensor.matmul(
                        out=po[:, :],
                        lhsT=w_sb[0:64, 640:672],
                        rhs=h5l[:, :],
                        start=True,
                        stop=True,
                    )
                    o = opool.tile([32, BLK], f16, tag="og", name="o_l")
                    act(o[:, :], po[:, :], bias_sb[0:32, 5:6], False)
                    nc.gpsimd.dma_start(
                        out=yt[0:32, ndbl * BLK : (ndbl + 1) * BLK], in_=o[:, :]
                    )
                    return
                pairs = dbl_pairs(g)
                rows = 64 * len(pairs)
                po = popool.tile([rows, BLK], f32, tag="po", name=f"po_{g}")
                for k, q in enumerate(pairs):
                    nc.tensor.matmul(
                        out=po[64 * k : 64 * (k + 1), :],
                        lhsT=w_sb[:, 640:704],
                        rhs=h5s[g][:, k * BLK : (k + 1) * BLK],
                        start=True,
                        stop=True,
                    )
                o = opool.tile([rows, BLK], f16, tag="og", name=f"o_{g}")
                act(o[:, :], po[:, :], bias_sb[0:rows, 5:6], False)
                nc.gpsimd.dma_start(
                    out=yt[0:rows, g * BLK : (g + 1) * BLK], in_=o[:, :]
                )

            # ---- Layers 3-5 sweeps (lone first), L6 trailing inside L5
            hcur = h2s
            hlp = h2l
            for li in (1, 2, 3):
                wc = 128 + li * 128
                hnext = []
                hln = None
                if lone:
                    ph = ppool.tile([128, 1024], f32, tag="ph", name=f"ph{li+2}_l")
                    nc.tensor.matmul(
                        out=ph[0:64, 0:BLK],
                        lhsT=w_sb[0:64, wc : wc + 64],
                        rhs=hlp[:, :],
                        start=True,
                        stop=True,
                    )
                    hln = hpool.tile(
                        [64, BLK], f16, tag=f"hl{li+2}", name=f"h{li+2}_l", bufs=1
                    )
                    act(hln[:, :], ph[0:64, 0:BLK], bias_sb[0:64, li + 1 : li + 2], True)
                pend6 = []
                if li == 3 and lone:
                    pend6.append(-1)  # lone L6 after first group's L5
                for d in range(ndbl):
                    ph = ppool.tile([128, 1024], f32, tag="ph", name=f"ph{li+2}_{d}")
                    w = len(dbl_pairs(d)) * BLK
                    for k, p in enumerate(dbl_pairs(d)):
                        co = k * BLK
                        nc.tensor.matmul(
                            out=ph[:, co : co + BLK],
                            lhsT=w_sb[:, wc : wc + 128],
                            rhs=hcur[d][:, co : co + BLK],
                            start=True,
                            stop=True,
                        )
                    h = hpool.tile([128, w], f16, tag=f"h{li+2}", name=f"h{li+2}_{d}")
                    act(h[:, :], ph[:, 0:w], bias_sb[:, li + 1 : li + 2], True)
                    hnext.append(h)
                    if li == 3:
                        # emit pending L6 groups (lagging one L5 group)
                        for g in pend6:
                            emit_l6(g, hnext, hln)
                        pend6 = [d]
                if li == 3:
                    for g in pend6:
                        emit_l6(g, hnext, hln)
                hcur = hnext
                hlp = hln

    nc.compile()
    return nc


def _get_program(nb: int):
    if nb not in _PROGRAM_CACHE:
        _PROGRAM_CACHE[nb] = _build_program(nb)
    return _PROGRAM_CACHE[nb]


def _prepare(state, rm_state, W1, b1, W2, b2, W3, b3, W4, b4, W5, b5, W6, b6):
    state = np.ascontiguousarray(np.asarray(state, dtype=np.float32))
    rm = np.asarray(rm_state).reshape(-1).astype(np.int64)
    Ws = [np.asarray(w, dtype=np.float32) for w in (W1, W2, W3, W4, W5, W6)]
    bs = [np.asarray(b, dtype=np.float32) for b in (b1, b2, b3, b4, b5, b6)]
    B = state.shape[0]
    X = state.reshape(B, D)

    # ---- host-side routing: all rows of expert k go to core k
    order = np.argsort(rm, kind="stable")
    counts = np.bincount(rm, minlength=E)
    nb = max(2, math.ceil(counts.max() / BLK))
    C = nb * BLK
    npair = nb // 2
    lone = nb % 2
    ndbl = (npair + 1) // 2
    ngrp = ndbl + lone
    csum = np.zeros(E, dtype=np.int64)
    csum[1:] = np.cumsum(counts)[:-1]
    sorted_expert = rm[order]
    pos_sorted = sorted_expert * C + (np.arange(B) - csum[sorted_expert])

    Xp = np.zeros((E * C, D), np.float16)
    Xp[pos_sorted] = X[order].astype(np.float16)

    W16 = [w.astype(np.float16) for w in Ws]

    in_maps = []
    for core in range(E):
        xt = Xp[core * C : (core + 1) * C].T  # [D, C] fp16 view
        # pairs: interleave the two 128-row halves per pair -> [128, 2048]
        parts = [
            xt[:, : npair * 1024]
            .reshape(2, 128, npair, 2 * BLK)
            .transpose(1, 2, 0, 3)
            .reshape(128, npair * 4 * BLK)
        ]
        if lone:
            xl = xt[:, npair * 1024 :].reshape(2, 128, BLK)
            parts.append(xl[0])
            parts.append(xl[1])
        xint = np.ascontiguousarray(np.concatenate(parts, axis=1))

        wh = np.zeros((128, WCOLS), np.float16)
        wh[:, 0:H] = W16[0][core, 0:128, :]
        wh[:, H : 2 * H] = W16[0][core, 128:256, :]
        for li in range(4):
            wc = 128 + li * 128
            wh[0:64, wc : wc + H] = W16[li + 1][core]
            wh[64:128, wc + H : wc + 128] = W16[li + 1][core]
        wh[0:64, 640 : 640 + A] = W16[5][core]
        wh[64:128, 672 : 672 + A] = W16[5][core]

        bh = np.zeros((128, 6), np.float32)
        for li in range(5):
            bh[0:64, li] = bs[li][core]
            bh[64:128, li] = bs[li][core]
        for r0 in (0, 32, 64, 96):
            bh[r0 : r0 + A, 5] = bs[5][core]

        in_maps.append({"xall": xint, "wt": wh, "bias": bh})

    meta = dict(
        B=B,
        C=C,
        nb=nb,
        npair=npair,
        lone=lone,
        ndbl=ndbl,
        ngrp=ngrp,
        order=order,
        pos_sorted=pos_sorted,
    )
    return in_maps, meta


def _finalize(results, meta):
    """results: list (per core) of dicts with 'yt' [128, ngrp*BLK] fp16."""
    B, C, nb, npair, lone, ndbl = (
        meta[k] for k in ("B", "C", "nb", "npair", "lone", "ndbl")
    )
    Yp = np.zeros((E * C, A), np.float32)
    for core in range(E):
        ytc = results[core]["yt"].astype(np.float32)
        for g in range(ndbl):
            cols = slice(g * BLK, (g + 1) * BLK)
            for k, q in enumerate((2 * g, 2 * g + 1)):
                if q >= npair:
                    continue
                dst = core * C + 2 * q * BLK
                r0 = 64 * k
                Yp[dst : dst + BLK] = ytc[r0 : r0 + A, cols].T
                Yp[dst + BLK : dst + 2 * BLK] = ytc[r0 + 32 : r0 + 32 + A, cols].T
        if lone:
            cols = slice(ndbl * BLK, (ndbl + 1) * BLK)
            dst = core * C + (nb - 1) * BLK
            Yp[dst : dst + BLK] = ytc[0:A, cols].T

    y = np.zeros((B, A), np.float32)
    y[meta["order"]] = Yp[meta["pos_sorted"]]
    return y


def kernel(state, rm_state, W1, b1, W2, b2, W3, b3, W4, b4, W5, b5, W6, b6):
    global LAST_RESULTS
    from concourse.bass_utils import run_bass_kernel_spmd

    in_maps, meta = _prepare(
        state, rm_state, W1, b1, W2, b2, W3, b3, W4, b4, W5, b5, W6, b6
    )
    nc = _get_program(meta["nb"])
    trace = bool(os.environ.get("KERNEL_TRACE"))
    res = run_bass_kernel_spmd(nc, in_maps, core_ids=list(range(NCORES)), trace=trace)
    LAST_RESULTS = res
    return _finalize(res.results, meta)


# revision 15
# speedup vs baseline: 1.1589x; 1.0152x over previous
"""MoE-routed DeepQNetwork kernel for 8x Trainium2 NeuronCores.

Problem: B=65536 rows, each routed to one of E=8 expert MLPs
(256 -> 64 -> 64 -> 64 -> 64 -> 64 -> 18, ReLU between layers).

Strategy v3 (expert-per-core sharding):
  E == NCORES and the routing is near-uniform (~8192 rows/expert), so core k
  owns ALL rows of expert k, padded to a uniform C = nb*512 columns (nb may
  be odd: pairs of 512-row blocks plus an optional lone block). Every core
  runs the same static program with a SINGLE expert's weights (~180 KB).

  Device (per core, SPMD):
  - PE warm-up: a burst of tiny matmuls right after the preamble releases
    the HAM clock gate (PE idles at 1.2 GHz until ~3.4us of activity) so the
    real matmuls run at 2.4 GHz.
  - x^T arrives as [256, C] fp16; pair p gets a [128, 2048] chunk (two
    blocks x two 128-row input halves). Even pairs + lone block stream on
    the sync HW-DGE queue, odd pairs + weights + bias on the gpsimd SW-DGE
    queue, so two descriptor rings fill SBUF concurrently; pair 0 is split
    into four [128, 512] chunks so the first matmul starts as early as
    possible.
  - L1 per pair: 4 matmuls on PE column groups (block even -> PSUM rows
    0:64, odd -> 64:128), contraction 256 over two accumulating chunks.
    L2-5: one [128,128] block-diag matmul per pair (the same 64x64 weight
    on both diagonals). PSUM/activation tiles cover TWO pairs ([128,1024],
    2 PSUM banks) so each ReLU+bias instruction drains four blocks; the L2
    sweep is interleaved into the x-DMA-paced L1 sweep to keep the PE fed.
    L6 stacks two pairs into one [128,512] PSUM bank (rows 0:18/32:50/
    64:82/96:114). The lone block runs first in every sweep on half-width
    tiles so it never sits in the serial tail.
  - fp32 accumulation in PSUM; ReLU+bias alternates VectorE/ScalarE;
    outputs leave as fp16 via gpsimd-issued DMAs per 4-block group.

  Host: unsort the fp16 outputs back to row order, cast to fp32.
"""

import math
import os

import numpy as np

E = 8
D = 256
H = 64
A = 18
NCORES = 8
BLK = 512  # rows per block (matmul moving-operand free dim / PSUM bank cols)
NWARM = 4  # PE warm-up matmuls bridging preamble-end to first x arrival

# per-core weight tile [128, WCOLS] fp16 column layout:
#   [0:64)    W1 chunk0 (input dims 0:128)
#   [64:128)  W1 chunk1 (input dims 128:256)
#   [128+128*li : 256+128*li) for li in 0..3: layer 2+li block-diag [128,128]
#             ([0:64,0:64] = W, [64:128,64:128] = W)
#   [640:704) W6 block-diag: [0:64, 0:18] = W6, [64:128, 32:50] = W6
WCOLS = 704

_PROGRAM_CACHE: dict = {}
LAST_RESULTS = None  # test harness can read timing/profile info from here


def _build_program(nb: int):
    """Build the SPMD bass program for nb 512-row blocks per core."""
    import concourse.mybir as mybir
    import concourse.tile as tile
    from concourse import bacc

    f32 = mybir.dt.float32
    f16 = mybir.dt.float16
    Relu = mybir.ActivationFunctionType.Relu
    add = mybir.AluOpType.add
    amax = mybir.AluOpType.max

    npair = nb // 2
    lone = nb % 2  # trailing unpaired block
    ndbl = (npair + 1) // 2  # two-pair tile groups (last may hold one pair)
    ngrp = ndbl + lone  # output column groups in yt

    nc = bacc.Bacc("TRN2")
    xall = nc.declare_dram_parameter(
        "xall", [128, npair * 2048 + lone * 1024], f16, isOutput=False
    )
    wt = nc.declare_dram_parameter("wt", [128, WCOLS], f16, isOutput=False)
    # bias cols 0:5 = b1..b5 (rows 0:64 == rows 64:128); col 5 = b6 at rows
    # 0:18 / 32:50 / 64:82 / 96:114
    bias = nc.declare_dram_parameter("bias", [128, 6], f32, isOutput=False)
    yt = nc.declare_dram_parameter("yt", [128, ngrp * BLK], f16, isOutput=True)

    act_flip = 0

    with tile.TileContext(nc) as tc:
        with (
            tc.tile_pool(name="wpool", bufs=1) as wpool,
            tc.tile_pool(name="xpool", bufs=npair + 4) as xpool,
            tc.tile_pool(name="hpool", bufs=ndbl) as hpool,
            tc.tile_pool(name="opool", bufs=3) as opool,
            tc.tile_pool(name="ppool", bufs=3, space="PSUM") as ppool,
            tc.tile_pool(name="popool", bufs=2, space="PSUM") as popool,
        ):
            def dbl_pairs(d):
                return [q for q in (2 * d, 2 * d + 1) if q < npair]

            # ---- PE warm-up source (memset, no DMA dependence)
            warm_src = wpool.tile([1, BLK], f16, name="warm_src", tag="ws", bufs=1)
            nc.vector.memset(warm_src[:, :], 0.0)

            # ---- DMA issue. Splitting x across two descriptor rings measured
            # SLOWER (early aggregate rate collapsed), so ALL x rides the sync
            # HW-DGE ring in exact consumption order; the tiny weight/bias
            # tensors ride the scalar HW-DGE ring in parallel, and gpsimd
            # keeps only the output stores.
            w_sb = wpool.tile([128, WCOLS], f16, name="w_sb", tag="w", bufs=1)
            nc.scalar.dma_start(out=w_sb[:, :], in_=wt[:, :])
            bias_sb = wpool.tile([128, 6], f32, name="bias_sb", tag="bias", bufs=1)
            nc.scalar.dma_start(out=bias_sb[:, :], in_=bias[:, :])

            # head: lone block first (smallest unit -> earliest real matmul),
            # then pair 0 as four [128,512] chunks in first-use order
            # (b0,c0) (b0,c1) (b1,c0) (b1,c1), then pair 1; the remaining
            # dbl groups arrive as single [128,4096] 1MB chunks (bigger
            # descriptors sustain a higher DMA rate, and the dependency
            # granularity matches the compute unit exactly).
            xl = None
            if lone:
                xl = xpool.tile([128, 1024], f16, tag="xl", name="xlone", bufs=1)
                nc.sync.dma_start(
                    out=xl[:, :], in_=xall[:, npair * 2048 : npair * 2048 + 1024]
                )

            p0 = []
            for i, off in enumerate((0, 1024, 512, 1536)):
                t = xpool.tile([128, BLK], f16, tag="x0", name=f"x0_{i}", bufs=4)
                nc.sync.dma_start(out=t[:, :], in_=xall[:, off : off + BLK])
                p0.append(t)

            xc1 = xpool.tile([128, 2048], f16, tag="xc1", name="xc_1", bufs=1)
            nc.sync.dma_start(out=xc1[:, :], in_=xall[:, 2048:4096])
            xds: list = [None] * ndbl
            for dd in range(1, ndbl):
                w = len(dbl_pairs(dd)) * 2048
                xd = xpool.tile([128, w], f16, tag="xd", name=f"xd_{dd}", bufs=ndbl)
                nc.sync.dma_start(
                    out=xd[:, :],
                    in_=xall[:, 2 * dd * 2048 : 2 * dd * 2048 + w],
                )
                xds[dd] = xd

            def x_rhs(p, blk, c):
                if p == 0:
                    return p0[2 * blk + c][:, :]
                if p == 1:
                    return xc1[:, c * 1024 + blk * BLK : c * 1024 + (blk + 1) * BLK]
                xd = xds[p // 2]
                off = (p % 2) * 2048 + c * 1024 + blk * BLK
                return xd[:, off : off + BLK]

            # ---- PE warm-up burst (writes cycle the ph ring, never read)
            for i in range(NWARM):
                pw = ppool.tile([128, 1024], f32, tag="ph", name=f"warm_{i}")
                nc.tensor.matmul(
                    out=pw[0:64, 0:BLK],
                    lhsT=warm_src[0:1, 0:64],
                    rhs=warm_src[0:1, :],
                    start=True,
                    stop=True,
                )

            def act(out_ap, in_ap, bias_ap, relu):
                nonlocal act_flip
                act_flip ^= 1
                if act_flip:
                    if relu:
                        nc.vector.tensor_scalar(
                            out_ap, in_ap, bias_ap, 0.0, op0=add, op1=amax
                        )
                    else:
                        nc.vector.tensor_scalar(
                            out_ap, in_ap, bias_ap, None, op0=add
                        )
                else:
                    if relu:
                        nc.scalar.activation(out_ap, in_ap, Relu, bias=bias_ap)
                    else:
                        nc.scalar.add(out_ap, in_ap, bias_ap)

            def emit_fill(i):
                # keep-warm filler: the HAM clock gate drops the PE back to
                # 1.2 GHz after ~3.4us of idle, so bridge early x-DMA gaps
                pw = ppool.tile([128, 1024], f32, tag="ph", name=f"fill_{i}")
                nc.tensor.matmul(
                    out=pw[0:64, 0:BLK],
                    lhsT=warm_src[0:1, 0:64],
                    rhs=warm_src[0:1, :],
                    start=True,
                    stop=True,
                )

            h1s = [None] * npair
            h1l = None

            def emit_l1(d):
                nonlocal h1l
                if d == -1:  # lone block
                    phl = ppool.tile([128, 1024], f32, tag="ph", name="ph1_l")
                    for c in (0, 1):
                        nc.tensor.matmul(
                            out=phl[0:64, 0:BLK],
                            lhsT=w_sb[:, c * H : (c + 1) * H],
                            rhs=xl[:, c * BLK : (c + 1) * BLK],
                            start=(c == 0),
                            stop=(c == 1),
                        )
                    h1l = hpool.tile([64, BLK], f16, tag="hl1", name="h1_l", bufs=1)
                    act(h1l[:, :], phl[0:64, 0:BLK], bias_sb[0:64, 0:1], True)
                    return
                ph1 = ppool.tile([128, 1024], f32, tag="ph", name=f"ph1_{d}")
                for k, p in enumerate(dbl_pairs(d)):
                    co = k * BLK
                    for blk, colr in ((0, slice(0, 64)), (1, slice(64, 128))):
                        for c in (0, 1):
                            nc.tensor.matmul(
                                out=ph1[colr, co : co + BLK],
                                lhsT=w_sb[:, c * H : (c + 1) * H],
                                rhs=x_rhs(p, blk, c),
                                start=(c == 0),
                                stop=(c == 1),
                            )
                    # per-pair activation: finer PSUM-drain granularity keeps
                    # the x-DMA-paced phase from stalling on act latency
                    h1 = hpool.tile(
                        [128, BLK], f16, tag="h1", name=f"h1_{p}", bufs=npair
                    )
                    act(h1[:, :], ph1[:, co : co + BLK], bias_sb[:, 0:1], True)
                    h1s[p] = h1

            h2s = [None] * ndbl
            h2l = None

            def emit_l2(d):
                nonlocal h2l
                if d == -1:  # lone block
                    ph = ppool.tile([128, 1024], f32, tag="ph", name="ph2_l")
                    nc.tensor.matmul(
                        out=ph[0:64, 0:BLK],
                        lhsT=w_sb[0:64, 128:192],
                        rhs=h1l[:, :],
                        start=True,
                        stop=True,
                    )
                    h2l = hpool.tile([64, BLK], f16, tag="hl2", name="h2_l", bufs=1)
                    act(h2l[:, :], ph[0:64, 0:BLK], bias_sb[0:64, 1:2], True)
                    return
                ph = ppool.tile([128, 1024], f32, tag="ph", name=f"ph2_{d}")
                w = len(dbl_pairs(d)) * BLK
                for k, p in enumerate(dbl_pairs(d)):
                    co = k * BLK
                    nc.tensor.matmul(
                        out=ph[:, co : co + BLK],
                        lhsT=w_sb[:, 128:256],
                        rhs=h1s[p][:, :],
                        start=True,
                        stop=True,
                    )
                h2 = hpool.tile([128, w], f16, tag="h2", name=f"h2_{d}")
                act(h2[:, :], ph[:, 0:w], bias_sb[:, 1:2], True)
                h2s[d] = h2

            # L1/L2 processing order = DMA arrival order: lone block first
            # (smallest unit), then the dbl groups; L2 lags two units behind
            units = ([-1] if lone else []) + list(range(ndbl))
            l2i = 0
            for i, u in enumerate(units):
                emit_l1(u)
                if i >= 2:
                    emit_l2(units[l2i])
                    l2i += 1
            while l2i < len(units):
                emit_l2(units[l2i])
                l2i += 1

            def emit_l6(g, h5s, h5l):
                # group g = pairs 2g (PSUM rows 0:64) and 2g+1 (64:128);
                # g == -1 is the lone block (rows 0:32, output col ndbl)
                if g == -1:
                    po = popool.tile([32, BLK], f32, tag="po", name="po_l")
                    nc.tensor.matmul(
                        out=po[:, :],
                        lhsT=w_sb[0:64, 640:672],
                        rhs=h5l[:, :],
                        start=True,
                        stop=True,
                    )
                    o = opool.tile([32, BLK], f16, tag="og", name="o_l")
                    act(o[:, :], po[:, :], bias_sb[0:32, 5:6], False)
                    nc.gpsimd.dma_start(
                        out=yt[0:32, ndbl * BLK : (ndbl + 1) * BLK], in_=o[:, :]
                    )
                    return
                pairs = dbl_pairs(g)
                rows = 64 * len(pairs)
                po = popool.tile([rows, BLK], f32, tag="po", name=f"po_{g}")
                for k, q in enumerate(pairs):
                    nc.tensor.matmul(
                        out=po[64 * k : 64 * (k + 1), :],
                        lhsT=w_sb[:, 640:704],
                        rhs=h5s[g][:, k * BLK : (k + 1) * BLK],
                        start=True,
                        stop=True,
                    )
                o = opool.tile([rows, BLK], f16, tag="og", name=f"o_{g}")
                act(o[:, :], po[:, :], bias_sb[0:rows, 5:6], False)
                nc.gpsimd.dma_start(
                    out=yt[0:rows, g * BLK : (g + 1) * BLK], in_=o[:, :]
                )

            # ---- Layers 3-5 sweeps (lone first), L6 trailing inside L5
            hcur = h2s
            hlp = h2l
            for li in (1, 2, 3):
                wc = 128 + li * 128
                hnext = []
                hln = None
                if lone:
                    ph = ppool.tile([128, 1024], f32, tag="ph", name=f"ph{li+2}_l")
                    nc.tensor.matmul(
                        out=ph[0:64, 0:BLK],
                        lhsT=w_sb[0:64, wc : wc + 64],
                        rhs=hlp[:, :],
                        start=True,
                        stop=True,
                    )
                    hln = hpool.tile(
                        [64, BLK], f16, tag=f"hl{li+2}", name=f"h{li+2}_l", bufs=1
                    )
                    act(hln[:, :], ph[0:64, 0:BLK], bias_sb[0:64, li + 1 : li + 2], True)
                pend6 = []
                if li == 3 and lone:
                    pend6.append(-1)  # lone L6 after first group's L5
                for d in range(ndbl):
                    ph = ppool.tile([128, 1024], f32, tag="ph", name=f"ph{li+2}_{d}")
                    w = len(dbl_pairs(d)) * BLK
                    for k, p in enumerate(dbl_pairs(d)):
                        co = k * BLK
                        nc.tensor.matmul(
                            out=ph[:, co : co + BLK],
                            lhsT=w_sb[:, wc : wc + 128],
                            rhs=hcur[d][:, co : co + BLK],
                            start=True,
                            stop=True,
                        )
                    h = hpool.tile([128, w], f16, tag=f"h{li+2}", name=f"h{li+2}_{d}")
                    act(h[:, :], ph[:, 0:w], bias_sb[:, li + 1 : li + 2], True)
                    hnext.append(h)
                    if li == 3:
                        # emit pending L6 groups (lagging one L5 group)
                        for g in pend6:
                            emit_l6(g, hnext, hln)
                        pend6 = [d]
                if li == 3:
                    for g in pend6:
                        emit_l6(g, hnext, hln)
                hcur = hnext
                hlp = hln

    nc.compile()
    return nc


def _get_program(nb: int):
    if nb not in _PROGRAM_CACHE:
        _PROGRAM_CACHE[nb] = _build_program(nb)
    return _PROGRAM_CACHE[nb]


def _prepare(state, rm_state, W1, b1, W2, b2, W3, b3, W4, b4, W5, b5, W6, b6):
    state = np.ascontiguousarray(np.asarray(state, dtype=np.float32))
    rm = np.asarray(rm_state).reshape(-1).astype(np.int64)
    Ws = [np.asarray(w, dtype=np.float32) for w in (W1, W2, W3, W4, W5, W6)]
    bs = [np.asarray(b, dtype=np.float32) for b in (b1, b2, b3, b4, b5, b6)]
    B = state.shape[0]
    X = state.reshape(B, D)

    # ---- host-side routing: all rows of expert k go to core k
    order = np.argsort(rm, kind="stable")
    counts = np.bincount(rm, minlength=E)
    nb = max(2, math.ceil(counts.max() / BLK))
    C = nb * BLK
    npair = nb // 2
    lone = nb % 2
    ndbl = (npair + 1) // 2
    ngrp = ndbl + lone
    csum = np.zeros(E, dtype=np.int64)
    csum[1:] = np.cumsum(counts)[:-1]
    sorted_expert = rm[order]
    pos_sorted = sorted_expert * C + (np.arange(B) - csum[sorted_expert])

    Xp = np.zeros((E * C, D), np.float16)
    Xp[pos_sorted] = X[order].astype(np.float16)

    W16 = [w.astype(np.float16) for w in Ws]

    in_maps = []
    for core in range(E):
        xt = Xp[core * C : (core + 1) * C].T  # [D, C] fp16 view
        # pairs: interleave the two 128-row halves per pair -> [128, 2048]
        parts = [
            xt[:, : npair * 1024]
            .reshape(2, 128, npair, 2 * BLK)
            .transpose(1, 2, 0, 3)
            .reshape(128, npair * 4 * BLK)
        ]
        if lone:
            xl = xt[:, npair * 1024 :].reshape(2, 128, BLK)
            parts.append(xl[0])
            parts.append(xl[1])
        xint = np.ascontiguousarray(np.concatenate(parts, axis=1))

        wh = np.zeros((128, WCOLS), np.float16)
        wh[:, 0:H] = W16[0][core, 0:128, :]
        wh[:, H : 2 * H] = W16[0][core, 128:256, :]
        for li in range(4):
            wc = 128 + li * 128
            wh[0:64, wc : wc + H] = W16[li + 1][core]
            wh[64:128, wc + H : wc + 128] = W16[li + 1][core]
        wh[0:64, 640 : 640 + A] = W16[5][core]
        wh[64:128, 672 : 672 + A] = W16[5][core]

        bh = np.zeros((128, 6), np.float32)
        for li in range(5):
            bh[0:64, li] = bs[li][core]
            bh[64:128, li] = bs[li][core]
        for r0 in (0, 32, 64, 96):
            bh[r0 : r0 + A, 5] = bs[5][core]

        in_maps.append({"xall": xint, "wt": wh, "bias": bh})

    meta = dict(
        B=B,
        C=C,
        nb=nb,
        npair=npair,
        lone=lone,
        ndbl=ndbl,
        ngrp=ngrp,
        order=order,
        pos_sorted=pos_sorted,
    )
    return in_maps, meta


def _finalize(results, meta):
    """results: list (per core) of dicts with 'yt' [128, ngrp*BLK] fp16."""
    B, C, nb, npair, lone, ndbl = (
        meta[k] for k in ("B", "C", "nb", "npair", "lone", "ndbl")
    )
    Yp = np.zeros((E * C, A), np.float32)
    for core in range(E):
        ytc = results[core]["yt"].astype(np.float32)
        for g in range(ndbl):
            cols = slice(g * BLK, (g + 1) * BLK)
            for k, q in enumerate((2 * g, 2 * g + 1)):
                if q >= npair:
                    continue
                dst = core * C + 2 * q * BLK
                r0 = 64 * k
                Yp[dst : dst + BLK] = ytc[r0 : r0 + A, cols].T
                Yp[dst + BLK : dst + 2 * BLK] = ytc[r0 + 32 : r0 + 32 + A, cols].T
        if lone:
            cols = slice(ndbl * BLK, (ndbl + 1) * BLK)
            dst = core * C + (nb - 1) * BLK
            Yp[dst : dst + BLK] = ytc[0:A, cols].T

    y = np.zeros((B, A), np.float32)
    y[meta["order"]] = Yp[meta["pos_sorted"]]
    return y


def kernel(state, rm_state, W1, b1, W2, b2, W3, b3, W4, b4, W5, b5, W6, b6):
    global LAST_RESULTS
    from concourse.bass_utils import run_bass_kernel_spmd

    in_maps, meta = _prepare(
        state, rm_state, W1, b1, W2, b2, W3, b3, W4, b4, W5, b5, W6, b6
    )
    nc = _get_program(meta["nb"])
    trace = bool(os.environ.get("KERNEL_TRACE"))
    res = run_bass_kernel_spmd(nc, in_maps, core_ids=list(range(NCORES)), trace=trace)
    LAST_RESULTS = res
    return _finalize(res.results, meta)


# revision 16
# speedup vs baseline: 1.1876x; 1.0248x over previous
"""MoE-routed DeepQNetwork kernel for 8x Trainium2 NeuronCores.

Problem: B=65536 rows, each routed to one of E=8 expert MLPs
(256 -> 64 -> 64 -> 64 -> 64 -> 64 -> 18, ReLU between layers).

Strategy v6 (expert-per-core sharding, software-pipelined wavefront):
  E == NCORES and the routing is near-uniform (~8192 rows/expert), so core k
  owns ALL rows of expert k, padded to a uniform C = nb*512 columns (nb may
  be odd: pairs of 512-row blocks plus one lone block). Every core runs the
  same static program with a SINGLE expert's weights (~180 KB).

  Device (per core, SPMD):
  - A short burst of tiny matmuls right after the preamble releases the HAM
    clock gate (the PE idles at 1.2 GHz until ~3.4us of sustained activity)
    so real matmuls run at 2.4 GHz from the start.
  - x^T arrives as [256, C] fp16 on the sync HW-DGE ring in consumption
    order (pair0 halves, pair1, then 1MB two-pair chunks, lone block last);
    weights+bias ride the scalar HW-DGE ring concurrently. A single ordered
    ring measured fastest (splitting x across rings halved early bandwidth).
  - Compute is a depth-6 software pipeline over "units" (two-pair groups +
    the lone block): each wave emits L6/L5/L4/L3/L2 of progressively older
    units before L1 of the newest, so the PE fills x-DMA wait time with
    deeper-layer work and output stores spread across the whole run instead
    of bunching in a serial tail.
  - L1 per pair: 4 matmuls on PE column groups (block even -> PSUM rows
    0:64, odd -> 64:128), contraction 256 over two accumulating chunks,
    per-pair ReLU+bias. L2-5: [128,128] block-diag matmuls per pair (same
    64x64 weight on both diagonals) into two-pair [128,1024] PSUM tiles
    drained by one ReLU+bias per two pairs. L6 stacks two pairs into one
    [128,512] PSUM bank (rows 0:18/32:50/64:82/96:114), bias-added and
    stored as fp16 via gpsimd-issued DMAs.

  Host: unsort the fp16 outputs back to row order, cast to fp32.
"""

import math
import os

import numpy as np

E = 8
D = 256
H = 64
A = 18
NCORES = 8
BLK = 512  # rows per block (matmul moving-operand free dim / PSUM bank cols)
NWARM = 4  # PE warm-up matmuls bridging preamble-end to first x arrival

# per-core weight tile [128, WCOLS] fp16 column layout:
#   [0:64)    W1 chunk0 (input dims 0:128)
#   [64:128)  W1 chunk1 (input dims 128:256)
#   [128+128*li : 256+128*li) for li in 0..3: layer 2+li block-diag [128,128]
#             ([0:64,0:64] = W, [64:128,64:128] = W)
#   [640:704) W6 block-diag: [0:64, 0:18] = W6, [64:128, 32:50] = W6
WCOLS = 704

_PROGRAM_CACHE: dict = {}
LAST_RESULTS = None  # test harness can read timing/profile info from here


def _build_program(nb: int):
    """Build the SPMD bass program for nb 512-row blocks per core."""
    import concourse.mybir as mybir
    import concourse.tile as tile
    from concourse import bacc

    f32 = mybir.dt.float32
    f16 = mybir.dt.float16
    Relu = mybir.ActivationFunctionType.Relu
    add = mybir.AluOpType.add
    amax = mybir.AluOpType.max

    npair = nb // 2
    lone = nb % 2  # trailing unpaired block
    ndbl = (npair + 1) // 2  # two-pair groups (last may hold one pair)
    ngrp = ndbl + lone  # output column groups in yt

    nc = bacc.Bacc("TRN2")
    xall = nc.declare_dram_parameter(
        "xall", [128, npair * 2048 + lone * 1024], f16, isOutput=False
    )
    wt = nc.declare_dram_parameter("wt", [128, WCOLS], f16, isOutput=False)
    # bias cols 0:5 = b1..b5 (rows 0:64 == rows 64:128); col 5 = b6 at rows
    # 0:18 / 32:50 / 64:82 / 96:114
    bias = nc.declare_dram_parameter("bias", [128, 6], f32, isOutput=False)
    yt = nc.declare_dram_parameter("yt", [128, ngrp * BLK], f16, isOutput=True)

    act_flip = 0

    with tile.TileContext(nc) as tc:
        with (
            tc.tile_pool(name="wpool", bufs=1) as wpool,
            tc.tile_pool(name="xpool", bufs=2) as xpool,
            tc.tile_pool(name="hpool", bufs=2) as hpool,
            tc.tile_pool(name="opool", bufs=3) as opool,
            tc.tile_pool(name="ppool", bufs=3, space="PSUM") as ppool,
            tc.tile_pool(name="popool", bufs=2, space="PSUM") as popool,
        ):
            def dbl_pairs(d):
                return [q for q in (2 * d, 2 * d + 1) if q < npair]

            # ---- PE warm-up source (memset, no DMA dependence)
            warm_src = wpool.tile([1, BLK], f16, name="warm_src", tag="ws", bufs=1)
            nc.vector.memset(warm_src[:, :], 0.0)

            # ---- DMA issue. All x on the sync HW-DGE ring in consumption
            # order; weights+bias on the scalar HW-DGE ring; outputs on
            # gpsimd. (Splitting x across rings measured slower.)
            w_sb = wpool.tile([128, WCOLS], f16, name="w_sb", tag="w", bufs=1)
            nc.scalar.dma_start(out=w_sb[:, :], in_=wt[:, :])
            bias_sb = wpool.tile([128, 6], f32, name="bias_sb", tag="bias", bufs=1)
            nc.scalar.dma_start(out=bias_sb[:, :], in_=bias[:, :])

            # pair0 as two [128,1024] half-chunks (first-needed first), then
            # pair1, then 1MB two-pair chunks; the lone block arrives last
            # (it is also processed last, so the pipeline drain is short).
            p0 = []
            for i in (0, 1):
                t = xpool.tile([128, 1024], f16, tag=f"x0h{i}", name=f"x0h{i}", bufs=1)
                nc.sync.dma_start(out=t[:, :], in_=xall[:, i * 1024 : (i + 1) * 1024])
                p0.append(t)
            xc1 = xpool.tile([128, 2048], f16, tag="xc1", name="xc_1", bufs=1)
            nc.sync.dma_start(out=xc1[:, :], in_=xall[:, 2048:4096])
            xds: list = [None] * ndbl
            for dd in range(1, ndbl):
                w = len(dbl_pairs(dd)) * 2048
                xd = xpool.tile([128, w], f16, tag=f"xd{dd}", name=f"xd_{dd}", bufs=1)
                nc.sync.dma_start(
                    out=xd[:, :], in_=xall[:, 2 * dd * 2048 : 2 * dd * 2048 + w]
                )
                xds[dd] = xd
            xl = None
            if lone:
                xl = xpool.tile([128, 1024], f16, tag="xl", name="xlone", bufs=1)
                nc.sync.dma_start(
                    out=xl[:, :], in_=xall[:, npair * 2048 : npair * 2048 + 1024]
                )

            def x_rhs(p, blk, c):
                if p == 0:
                    return p0[c][:, blk * BLK : (blk + 1) * BLK]
                if p == 1:
                    return xc1[:, c * 1024 + blk * BLK : c * 1024 + (blk + 1) * BLK]
                xd = xds[p // 2]
                off = (p % 2) * 2048 + c * 1024 + blk * BLK
                return xd[:, off : off + BLK]

            # ---- PE warm-up burst (writes cycle the ph ring, never read)
            for i in range(NWARM):
                pw = ppool.tile([128, 1024], f32, tag="ph", name=f"warm_{i}")
                nc.tensor.matmul(
                    out=pw[0:64, 0:BLK],
                    lhsT=warm_src[0:1, 0:64],
                    rhs=warm_src[0:1, :],
                    start=True,
                    stop=True,
                )

            def act(out_ap, in_ap, bias_ap, relu):
                nonlocal act_flip
                act_flip ^= 1
                if act_flip:
                    if relu:
                        nc.vector.tensor_scalar(
                            out_ap, in_ap, bias_ap, 0.0, op0=add, op1=amax
                        )
                    else:
                        nc.vector.tensor_scalar(out_ap, in_ap, bias_ap, None, op0=add)
                else:
                    if relu:
                        nc.scalar.activation(out_ap, in_ap, Relu, bias=bias_ap)
                    else:
                        nc.scalar.add(out_ap, in_ap, bias_ap)

            # h storage: layer 1 per pair (+ lone), layers 2-5 per dbl (+ lone)
            h1s = [None] * npair
            hdbl = {li: [None] * ndbl for li in (2, 3, 4, 5)}
            hlon = {}

            def emit_s1(u):
                if u == -1:
                    phl = ppool.tile([128, 1024], f32, tag="ph", name="ph1_l")
                    for c in (0, 1):
                        nc.tensor.matmul(
                            out=phl[0:64, 0:BLK],
                            lhsT=w_sb[:, c * H : (c + 1) * H],
                            rhs=xl[:, c * BLK : (c + 1) * BLK],
                            start=(c == 0),
                            stop=(c == 1),
                        )
                    hl = hpool.tile([64, BLK], f16, tag="hl1", name="h1_l", bufs=1)
                    act(hl[:, :], phl[0:64, 0:BLK], bias_sb[0:64, 0:1], True)
                    hlon[1] = hl
                    return
                ph1 = ppool.tile([128, 1024], f32, tag="ph", name=f"ph1_{u}")
                for k, p in enumerate(dbl_pairs(u)):
                    co = k * BLK
                    for blk, colr in ((0, slice(0, 64)), (1, slice(64, 128))):
                        for c in (0, 1):
                            nc.tensor.matmul(
                                out=ph1[colr, co : co + BLK],
                                lhsT=w_sb[:, c * H : (c + 1) * H],
                                rhs=x_rhs(p, blk, c),
                                start=(c == 0),
                                stop=(c == 1),
                            )
                    # per-pair activation: finer PSUM-drain granularity in
                    # the x-DMA-paced phase
                    h1 = hpool.tile(
                        [128, BLK], f16, tag=f"h1_{p}", name=f"h1_{p}", bufs=1
                    )
                    act(h1[:, :], ph1[:, co : co + BLK], bias_sb[:, 0:1], True)
                    h1s[p] = h1

            def emit_mid(li, u):
                # layer li in 2..5: [64 -> 64] block-diag
                wc = 128 + (li - 2) * 128
                bap_rows = slice(0, 64)
                if u == -1:
                    prev = hlon[1] if li == 2 else hlon[li - 1]
                    ph = ppool.tile([128, 1024], f32, tag="ph", name=f"ph{li}_l")
                    nc.tensor.matmul(
                        out=ph[0:64, 0:BLK],
                        lhsT=w_sb[0:64, wc : wc + 64],
                        rhs=prev[:, :],
                        start=True,
                        stop=True,
                    )
                    hl = hpool.tile(
                        [64, BLK], f16, tag=f"hl{li}", name=f"h{li}_l", bufs=1
                    )
                    act(hl[:, :], ph[0:64, 0:BLK], bias_sb[bap_rows, li - 1 : li], True)
                    hlon[li] = hl
                    return
                ph = ppool.tile([128, 1024], f32, tag="ph", name=f"ph{li}_{u}")
                w = len(dbl_pairs(u)) * BLK
                for k, p in enumerate(dbl_pairs(u)):
                    co = k * BLK
                    rhs = h1s[p][:, :] if li == 2 else hdbl[li - 1][u][:, co : co + BLK]
                    nc.tensor.matmul(
                        out=ph[:, co : co + BLK],
                        lhsT=w_sb[:, wc : wc + 128],
                        rhs=rhs,
                        start=True,
                        stop=True,
                    )
                h = hpool.tile([128, w], f16, tag=f"h{li}_{u}", name=f"h{li}_{u}", bufs=1)
                act(h[:, :], ph[:, 0:w], bias_sb[:, li - 1 : li], True)
                hdbl[li][u] = h

            def emit_s6(u):
                # L6 [64 -> 18]: group u = pairs 2u (PSUM rows 0:64) and
                # 2u+1 (rows 64:128); u == -1 = lone block (rows 0:32,
                # output column group ndbl)
                if u == -1:
                    po = popool.tile([32, BLK], f32, tag="po", name="po_l")
                    nc.tensor.matmul(
                        out=po[:, :],
                        lhsT=w_sb[0:64, 640:672],
                        rhs=hlon[5][:, :],
                        start=True,
                        stop=True,
                    )
                    o = opool.tile([32, BLK], f16, tag="og", name="o_l")
                    act(o[:, :], po[:, :], bias_sb[0:32, 5:6], False)
                    nc.gpsimd.dma_start(
                        out=yt[0:32, ndbl * BLK : (ndbl + 1) * BLK], in_=o[:, :]
                    )
                    return
                pairs = dbl_pairs(u)
                rows = 64 * len(pairs)
                po = popool.tile([rows, BLK], f32, tag="po", name=f"po_{u}")
                for k, q in enumerate(pairs):
                    nc.tensor.matmul(
                        out=po[64 * k : 64 * (k + 1), :],
                        lhsT=w_sb[:, 640:704],
                        rhs=hdbl[5][u][:, k * BLK : (k + 1) * BLK],
                        start=True,
                        stop=True,
                    )
                o = opool.tile([rows, BLK], f16, tag="og", name=f"o_{u}")
                act(o[:, :], po[:, :], bias_sb[0:rows, 5:6], False)
                nc.gpsimd.dma_start(
                    out=yt[0:rows, u * BLK : (u + 1) * BLK], in_=o[:, :]
                )

            # ---- depth-6 software pipeline: units in x-arrival order, lone
            # last; within a wave, deeper (older) stages first so an x wait
            # never starves ready work, and stage 2 lags stage 1 by 2 waves
            # so L1 activations have slack.
            units = list(range(ndbl)) + ([-1] if lone else [])
            nunits = len(units)
            lag = {1: 0, 2: 2, 3: 3, 4: 4, 5: 5, 6: 6}

            def emit_stage(s, u):
                if s == 1:
                    emit_s1(u)
                elif s == 6:
                    emit_s6(u)
                else:
                    emit_mid(s, u)

            for wave in range(nunits + lag[6]):
                for s in (6, 5, 4, 3, 2, 1):
                    i = wave - lag[s]
                    if 0 <= i < nunits:
                        emit_stage(s, units[i])

    nc.compile()
    return nc


def _get_program(nb: int):
    if nb not in _PROGRAM_CACHE:
        _PROGRAM_CACHE[nb] = _build_program(nb)
    return _PROGRAM_CACHE[nb]


def _prepare(state, rm_state, W1, b1, W2, b2, W3, b3, W4, b4, W5, b5, W6, b6):
    state = np.ascontiguousarray(np.asarray(state, dtype=np.float32))
    rm = np.asarray(rm_state).reshape(-1).astype(np.int64)
    Ws = [np.asarray(w, dtype=np.float32) for w in (W1, W2, W3, W4, W5, W6)]
    bs = [np.asarray(b, dtype=np.float32) for b in (b1, b2, b3, b4, b5, b6)]
    B = state.shape[0]
    X = state.reshape(B, D)

    # ---- host-side routing: all rows of expert k go to core k
    order = np.argsort(rm, kind="stable")
    counts = np.bincount(rm, minlength=E)
    nb = max(2, math.ceil(counts.max() / BLK))
    C = nb * BLK
    npair = nb // 2
    lone = nb % 2
    ndbl = (npair + 1) // 2
    ngrp = ndbl + lone
    csum = np.zeros(E, dtype=np.int64)
    csum[1:] = np.cumsum(counts)[:-1]
    sorted_expert = rm[order]
    pos_sorted = sorted_expert * C + (np.arange(B) - csum[sorted_expert])

    Xp = np.zeros((E * C, D), np.float16)
    Xp[pos_sorted] = X[order].astype(np.float16)

    W16 = [w.astype(np.float16) for w in Ws]

    in_maps = []
    for core in range(E):
        xt = Xp[core * C : (core + 1) * C].T  # [D, C] fp16 view
        # pairs: interleave the two 128-row halves per pair -> [128, 2048]
        parts = [
            xt[:, : npair * 1024]
            .reshape(2, 128, npair, 2 * BLK)
            .transpose(1, 2, 0, 3)
            .reshape(128, npair * 4 * BLK)
        ]
        if lone:
            xlh = xt[:, npair * 1024 :].reshape(2, 128, BLK)
            parts.append(xlh[0])
            parts.append(xlh[1])
        xint = np.ascontiguousarray(np.concatenate(parts, axis=1))

        wh = np.zeros((128, WCOLS), np.float16)
        wh[:, 0:H] = W16[0][core, 0:128, :]
        wh[:, H : 2 * H] = W16[0][core, 128:256, :]
        for li in range(4):
            wc = 128 + li * 128
            wh[0:64, wc : wc + H] = W16[li + 1][core]
            wh[64:128, wc + H : wc + 128] = W16[li + 1][core]
        wh[0:64, 640 : 640 + A] = W16[5][core]
        wh[64:128, 672 : 672 + A] = W16[5][core]

        bh = np.zeros((128, 6), np.float32)
        for li in range(5):
            bh[0:64, li] = bs[li][core]
            bh[64:128, li] = bs[li][core]
        for r0 in (0, 32, 64, 96):
            bh[r0 : r0 + A, 5] = bs[5][core]

        in_maps.append({"xall": xint, "wt": wh, "bias": bh})

    meta = dict(
        B=B,
        C=C,
        nb=nb,
        npair=npair,
        lone=lone,
        ndbl=ndbl,
        ngrp=ngrp,
        order=order,
        pos_sorted=pos_sorted,
    )
    return in_maps, meta


def _finalize(results, meta):
    """results: list (per core) of dicts with 'yt' [128, ngrp*BLK] fp16."""
    B, C, nb, npair, lone, ndbl = (
        meta[k] for k in ("B", "C", "nb", "npair", "lone", "ndbl")
    )
    Yp = np.zeros((E * C, A), np.float32)
    for core in range(E):
        ytc = results[core]["yt"].astype(np.float32)
        for g in range(ndbl):
            cols = slice(g * BLK, (g + 1) * BLK)
            for k, q in enumerate((2 * g, 2 * g + 1)):
                if q >= npair:
                    continue
                dst = core * C + 2 * q * BLK
                r0 = 64 * k
                Yp[dst : dst + BLK] = ytc[r0 : r0 + A, cols].T
                Yp[dst + BLK : dst + 2 * BLK] = ytc[r0 + 32 : r0 + 32 + A, cols].T
        if lone:
            cols = slice(ndbl * BLK, (ndbl + 1) * BLK)
            dst = core * C + (nb - 1) * BLK
            Yp[dst : dst + BLK] = ytc[0:A, cols].T

    y = np.zeros((B, A), np.float32)
    y[meta["order"]] = Yp[meta["pos_sorted"]]
    return y


def kernel(state, rm_state, W1, b1, W2, b2, W3, b3, W4, b4, W5, b5, W6, b6):
    global LAST_RESULTS
    from concourse.bass_utils import run_bass_kernel_spmd

    in_maps, meta = _prepare(
        state, rm_state, W1, b1, W2, b2, W3, b3, W4, b4, W5, b5, W6, b6
    )
    nc = _get_program(meta["nb"])
    trace = bool(os.environ.get("KERNEL_TRACE"))
    res = run_bass_kernel_spmd(nc, in_maps, core_ids=list(range(NCORES)), trace=trace)
    LAST_RESULTS = res
    return _finalize(res.results, meta)
